# revision 1
# baseline (speedup 1.0000x reference)
"""BLOOM attention block (fused QKV proj + causal alibi attention + dense
projection) on 8 Trainium2 NeuronCores.

Sharding: tensor-parallel over heads. Each core owns 4 of the 32 heads:
it computes those heads' Q/K/V projections (column-sharded W_qkv),
attention, and a partial dense output (row-sharded W_dense over the same
head channels). The host sums the 8 partial outputs and adds
b_dense + residual.

Device-side design notes:
  - Activations are kept transposed ([feature, token]) so every matmul
    contracts over the partition dim with no on-chip transposes. Attention
    scores are computed directly transposed (sT = kT.T @ qT) so exp()
    writes probs^T straight into SBUF for the PV matmul.
  - Matmul inputs are bf16 (full PE rate); accumulation is fp32. The Q/K
    projection runs in fp8 (x64 range lift): its output only shifts softmax
    logits by ~1e-3 against an alibi scale of ~1e2, so fp8 error is
    invisible — and the fp8 Wqk shard stays resident in SBUF.
  - Softmax needs no reduce_max: the exp shift is the host-precomputed
    -(running_max(alibi)+1) (softmax is shift invariant; |q.k/sqrt(hd)|<<1).
    alibi[k] is a per-partition scalar in the transposed layout; both are
    applied in exact fp32 by one DVE scalar_tensor_tensor per score chunk.
  - Row sums come from a ones^T @ probs^T matmul; 1/sum is partition-
    broadcast (GpSimd) and fused into the small ctx copy (DVE), exact fp32.
  - The causal mask is additive -30000 on the 128x128 diagonal blocks only;
    blocks strictly below the transposed diagonal are never computed.
  - Host-side DRAM layouts are pre-tiled so every big DMA reads 16-32 KiB
    per-partition-contiguous runs; DMA issue streams are split across the
    SP/ACT/GpSimd sequencers so slot-gated waits never block prefetches.
"""

import math

import numpy as np
import ml_dtypes

B, S, H, NH = 2, 1024, 4096, 32
HD = H // NH  # 128
T = B * S  # 2048 tokens
NCORES = 8
HPC = NH // NCORES  # 4 heads per core
INV = 1.0 / math.sqrt(HD)
BF16 = ml_dtypes.bfloat16
F8 = ml_dtypes.float8_e4m3
QK8_SCALE = 64.0  # fp8 range lift for hidden/Wqk; descaled after the matmul
QK8_DESCALE = 1.0 / (QK8_SCALE * QK8_SCALE)
MASKVAL = -30000.0

KO = H // 128  # 32 contraction subtiles over the hidden dim
TCH = 256  # token chunk in the projection phase
CT_QK = 2 * HPC  # 8 q/k channel tiles per core (q_h0,k_h0,q_h1,k_h1,...)
ITEMS = B * HPC  # 8 (batch, head) attention items per core
QT = S // 128  # 8 query tiles per item

# eT blocks (k_tile, q_tile) that the PV matmul reads but no transpose
# writes (strictly-above-diagonal inside each 512-wide q chunk).
ZERO_BLOCKS = [
    (kt, qi)
    for qc in range(2)
    for kt in range(4 * qc, 4 * qc + 4)
    for qi in range(4 * qc, 4 * qc + 4)
    if kt > qi
]

_cache: dict = {}


def _build_nc():
    """Build the (SPMD, per-core) Bass/Tile program. Same program runs on
    all 8 cores; only the input data differs per core."""
    import concourse.bass as bass
    import concourse.mybir as mybir
    import concourse.tile as tile
    from concourse import bacc

    dt = mybir.dt
    f32, bf16 = dt.float32, dt.bfloat16
    AF = mybir.ActivationFunctionType
    AX = mybir.AxisListType

    nc = bacc.Bacc("TRN2", debug=False, num_devices=NCORES)

    # pre-tiled (host-side) layouts: every DMA reads per-partition-contiguous
    # runs (16-32 KiB), which maximizes per-queue DMA throughput
    f8 = dt.float8e4
    hidc = nc.dram_tensor(
        "hidc", [T // TCH, 128, KO, TCH], bf16, kind="ExternalInput"
    ).ap()
    # fp8 copies for the Q/K projection only: the q.k logits are tiny
    # compared to alibi, so fp8 weight/activation error is negligible there.
    # This lets the whole Wqk shard stay resident in SBUF (no re-streaming).
    hid8c = nc.dram_tensor(
        "hid8c", [T // TCH, 128, KO, TCH], f8, kind="ExternalInput"
    ).ap()
    wqk8c = nc.dram_tensor(
        "wqk8c", [CT_QK // 2, 128, KO, 256], f8, kind="ExternalInput"
    ).ap()
    wvc = nc.dram_tensor("wvc", [128, KO, HPC * 128], bf16, kind="ExternalInput").ap()
    wdc = nc.dram_tensor(
        "wdc", [H // 256, 128, HPC, 256], bf16, kind="ExternalInput"
    ).ap()
    bqk = nc.dram_tensor("bqk", [128, CT_QK], f32, kind="ExternalInput").ap()
    bv2 = nc.dram_tensor("bv2", [2, HPC * 128], bf16, kind="ExternalInput").ap()
    ones3 = nc.dram_tensor("ones3", [3, 128], bf16, kind="ExternalInput").ap()
    # additive score terms, exact fp32: alibi[k] is a per-partition scalar
    # in the transposed score layout; -(running_max(alibi[:q]) + 1) (the
    # static exp shift replacing a reduce_max) is partition-broadcast.
    alibik = nc.dram_tensor("alibik", [ITEMS, S], f32, kind="ExternalInput").ap()
    negcr = nc.dram_tensor("negcr", [ITEMS, S], f32, kind="ExternalInput").ap()
    # transposed causal diagonal blocks (additive MASKVAL)
    maskd = nc.dram_tensor("maskd", [QT, 128, 128], f32, kind="ExternalInput").ap()
    outT = nc.dram_tensor("outT", [H, T], bf16, kind="ExternalOutput").ap()

    maskd3 = maskd.rearrange("q p k -> p q k")

    with tile.TileContext(nc) as tc:
        with (
            tc.tile_pool(name="consts", bufs=1) as consts,
            tc.tile_pool(name="persist", bufs=1) as persist,
        ):
            bqk_sb = consts.tile([128, CT_QK], f32, tag="bqk")
            nc.gpsimd.dma_start(bqk_sb, bqk)
            bv2_sb = consts.tile([2, HPC * 128], bf16, tag="bv2")
            nc.gpsimd.dma_start(bv2_sb, bv2)
            ones3_sb = consts.tile([3, 128], bf16, tag="ones3")
            nc.gpsimd.dma_start(ones3_sb, ones3)
            ones2_sb = ones3_sb[:2, :]
            maskd_sb = consts.tile([128, QT, 128], f32, tag="maskd")
            nc.gpsimd.dma_start(maskd_sb, maskd3)

            # Long-lived per-core activations.
            qkT_t = persist.tile([128, CT_QK, T], bf16, tag="qkT")
            v_t = persist.tile([128, T // 128, HPC * 128], bf16, tag="v")
            ctxT_t = persist.tile([128, HPC, T], bf16, tag="ctxT")

            # ---- Phase A+B merged: one pass over hidden-state chunks
            # computes both the V projection ([token, ch] layout) and the
            # Q/K projection ([ch, token] layout). W_qk tiles are re-DMAed
            # per chunk (cheap); wv stays resident.
            with (
                tc.tile_pool(name="hidp", bufs=2) as hidp,
                tc.tile_pool(name="hid8p", bufs=2) as hid8p,
                tc.tile_pool(name="wvp", bufs=1) as wvp,
                tc.tile_pool(name="wqk8p", bufs=1) as wqk8p,
                tc.tile_pool(name="psA", bufs=4, space="PSUM") as psA,
            ):
                wv_sb = wvp.tile([128, KO, HPC * 128], bf16, tag="wv")
                wqk8_sb = wqk8p.tile([128, CT_QK // 2, KO, 256], f8, tag="wqk8")
                for tci in range(T // TCH):
                    # hid8 (q/k inputs) first and on SP, so the first matmul
                    # group's inputs transfer in parallel with the w tiles on
                    # the ACT queue; the bf16 copy (V inputs, needed later in
                    # the chunk) queues behind it.
                    hid8 = hid8p.tile([128, KO, TCH], f8, tag="hid8")
                    nc.sync.dma_start(hid8, hid8c[tci])
                    hid = hidp.tile([128, KO, TCH], bf16, tag="hid")
                    nc.sync.dma_start(hid, hidc[tci])
                    for cp in range(CT_QK // 2):
                        if tci == 0:
                            nc.scalar.dma_start(wqk8_sb[:, cp], wqk8c[cp])
                        for half in range(2):
                            ct = 2 * cp + half
                            ps = psA.tile([128, TCH], f32, tag="qk")
                            for ko in range(KO):
                                nc.tensor.matmul(
                                    ps,
                                    wqk8_sb[:, cp, ko, half * 128 : (half + 1) * 128],
                                    hid8[:, ko, :],
                                    start=(ko == 0),
                                    stop=(ko == KO - 1),
                                )
                            # descale fp8 product + bias-add + bf16 cast (DVE)
                            nc.vector.tensor_scalar(
                                out=qkT_t[:, ct, tci * TCH : (tci + 1) * TCH],
                                in0=ps,
                                scalar1=QK8_DESCALE,
                                scalar2=bqk_sb[:, ct : ct + 1],
                                op0=mybir.AluOpType.mult,
                                op1=mybir.AluOpType.add,
                            )
                    if tci == 0:
                        # deferred: needed only by the V matmuls below
                        nc.scalar.dma_start(wv_sb, wvc)
                    for tt in range(TCH // 128):
                        ps = psA.tile([128, 512], f32, tag="mm", bufs=2)
                        for ko in range(KO):
                            nc.tensor.matmul(
                                ps,
                                hid[:, ko, tt * 128 : (tt + 1) * 128],
                                wv_sb[:, ko, :],
                                start=(ko == 0),
                                stop=False,
                            )
                        # bias as a rank-2 update: [1;1]^T @ [bv_hi; bv_lo]
                        nc.tensor.matmul(
                            ps, ones2_sb, bv2_sb, start=False, stop=True
                        )
                        nc.vector.tensor_copy(
                            out=v_t[:, tci * (TCH // 128) + tt, :], in_=ps
                        )

            # ---- Phase C: attention per (batch, head) item.
            # Scores are computed DIRECTLY transposed: sT[k, q] = kT.T @ qT,
            # with alibi[k] and the per-row exp shift -c[q] folded in as a
            # rank-6 matmul update (3 bf16 terms each). exp() then writes
            # probs^T straight into SBUF — no PE transposes, no copies.
            # Row sums come from a ones^T @ eT matmul; 1/sum is broadcast
            # across partitions (GpSimd) and applied at the small ctx copy.
            with (
                tc.tile_pool(name="alp", bufs=4) as alp,
                tc.tile_pool(name="etp", bufs=3) as etp,
                tc.tile_pool(name="rcp", bufs=2) as rcp,
                tc.tile_pool(name="psS", bufs=6, space="PSUM") as psS,
                tc.tile_pool(name="psE", bufs=1, space="PSUM") as psE,
                tc.tile_pool(name="psA", bufs=1, space="PSUM") as psA,
            ):
                ones_col = consts.tile([128, 1], bf16, tag="ones_col")
                nc.gpsimd.memset(ones_col, 1.0)
                state: dict = {}

                def chunks_of(kt):
                    q0 = kt * 128
                    if q0 < 512:
                        return [(q0, 512), (512, S)]
                    return [(q0, S)]

                def item_setup(it):
                    b, hl = divmod(it, HPC)
                    alik = alp.tile([128, QT], f32, tag="alik")
                    nc.scalar.dma_start(
                        alik, alibik[it].rearrange("(kt p) -> p kt", p=128)
                    )
                    ncrow = alp.tile([1, S], f32, tag="ncrow")
                    nc.scalar.dma_start(ncrow, negcr[it][None, :])
                    ncb = alp.tile([128, S], f32, tag="ncb")
                    nc.gpsimd.partition_broadcast(ncb, ncrow)
                    eT = etp.tile([128, QT, S], bf16, tag="eT")
                    for kt, qi in ZERO_BLOCKS:
                        nc.gpsimd.memset(eT[:, kt, qi * 128 : (qi + 1) * 128], 0.0)
                    state[it] = dict(b=b, hl=hl, alik=alik, ncb=ncb, eT=eT)

                def score_stage(it, kt):
                    st = state[it]
                    b, hl = st["b"], st["hl"]
                    qTh = qkT_t[:, 2 * hl, b * S : (b + 1) * S]
                    kTh = qkT_t[:, 2 * hl + 1, b * S : (b + 1) * S]
                    eT = st["eT"]
                    for ci, (q0, q1) in enumerate(chunks_of(kt)):
                        ps = psS.tile([128, 512], f32, tag="s")
                        nc.tensor.matmul(
                            ps[:, : q1 - q0],
                            kTh[:, kt * 128 : (kt + 1) * 128],
                            qTh[:, q0:q1],
                            start=True,
                            stop=True,
                        )
                        # score += alibi[k] (per-partition) + negc[q] (bcast)
                        nc.vector.scalar_tensor_tensor(
                            out=ps[:, : q1 - q0],
                            in0=ps[:, : q1 - q0],
                            scalar=st["alik"][:, kt : kt + 1],
                            in1=st["ncb"][:, q0:q1],
                            op0=mybir.AluOpType.add,
                            op1=mybir.AluOpType.add,
                        )
                        if ci == 0:  # causal diagonal block: first 128 cols
                            nc.vector.tensor_add(
                                ps[:, :128], ps[:, :128], maskd_sb[:, kt, :]
                            )
                        nc.scalar.activation(
                            eT[:, kt, q0:q1],
                            ps[:, : q1 - q0],
                            AF.Exp,
                            bias=0.0,
                            scale=1.0,
                        )

                def sum_stage(it):
                    st = state[it]
                    eT = st["eT"]
                    rcrow = rcp.tile([1, S], f32, tag="rcrow")
                    for qc in range(2):
                        ktn = 4 * (qc + 1)
                        ps = psE.tile([1, 512], f32, tag="se")
                        for kt in range(ktn):
                            nc.tensor.matmul(
                                ps,
                                ones_col,
                                eT[:, kt, qc * 512 : (qc + 1) * 512],
                                start=(kt == 0),
                                stop=(kt == ktn - 1),
                            )
                        nc.vector.reciprocal(
                            rcrow[:, qc * 512 : (qc + 1) * 512], ps
                        )
                    rcb = rcp.tile([128, S], f32, tag="rcb")
                    nc.gpsimd.partition_broadcast(rcb, rcrow)
                    st["rcb"] = rcb

                def pv_stage(it):
                    st = state.pop(it)
                    b, hl, eT = st["b"], st["hl"], st["eT"]
                    for qc in range(2):
                        ktn = 4 * (qc + 1)
                        ps = psA.tile([128, 512], f32, tag="mm")
                        for kt in range(ktn):
                            nc.tensor.matmul(
                                ps,
                                v_t[:, b * 8 + kt, hl * 128 : (hl + 1) * 128],
                                eT[:, kt, qc * 512 : (qc + 1) * 512],
                                start=(kt == 0),
                                stop=(kt == ktn - 1),
                            )
                        # fused 1/rowsum normalization + bf16 cast
                        nc.vector.tensor_tensor(
                            out=ctxT_t[
                                :, hl, b * S + qc * 512 : b * S + (qc + 1) * 512
                            ],
                            in0=ps,
                            in1=st["rcb"][:, qc * 512 : (qc + 1) * 512],
                            op=mybir.AluOpType.mult,
                        )

                # Two items in lockstep: the PE always has the other item's
                # independent score matmuls while one item's add->exp chain
                # drains on DVE/ACT.
                for g in range(ITEMS // 2):
                    pair = (2 * g, 2 * g + 1)
                    for it in pair:
                        item_setup(it)
                    for kt in range(QT):
                        for it in pair:
                            score_stage(it, kt)
                    for it in pair:
                        sum_stage(it)
                    for it in pair:
                        pv_stage(it)

            # ---- Phase D: partial dense, outT[o, t] = sum_c Wd[c, o] ctx[t, c]
            with (
                tc.tile_pool(name="wdp", bufs=3) as wdp,
                tc.tile_pool(name="outp", bufs=4) as outp,
                tc.tile_pool(name="psA", bufs=4, space="PSUM") as psA,
            ):
                for op_ in range(H // 256):
                    wdt = wdp.tile([128, HPC, 256], bf16, tag="wd")
                    nc.scalar.dma_start(wdt, wdc[op_])
                    for half in range(2):
                        ot = 2 * op_ + half
                        ob = outp.tile([128, T], bf16, tag="ob")
                        for tcd in range(T // 512):
                            ps = psA.tile([128, 512], f32, tag="mm")
                            for ko in range(HPC):
                                nc.tensor.matmul(
                                    ps,
                                    wdt[:, ko, half * 128 : (half + 1) * 128],
                                    ctxT_t[:, ko, tcd * 512 : (tcd + 1) * 512],
                                    start=(ko == 0),
                                    stop=(ko == HPC - 1),
                                )
                            nc.vector.tensor_copy(
                                out=ob[:, tcd * 512 : (tcd + 1) * 512], in_=ps
                            )
                        nc.sync.dma_start(outT[ot * 128 : (ot + 1) * 128, :], ob)
    nc.compile()
    return nc


def _get_nc():
    if "nc" not in _cache:
        _cache["nc"] = _build_nc()
    return _cache["nc"]


def make_in_maps(
    hidden_states, alibi, attention_mask, W_qkv, b_qkv, W_dense
) -> list[dict]:
    """Host-side sharding/preprocessing: per-core input dicts."""
    hs = np.asarray(hidden_states, np.float32)
    al = np.asarray(alibi, np.float32)
    am = np.asarray(attention_mask).astype(bool)
    wqkv = np.asarray(W_qkv, np.float32)
    bqkv = np.asarray(b_qkv, np.float32)
    wdn = np.asarray(W_dense, np.float32)

    hidT_b = hs.reshape(T, H).T.astype(BF16)  # [H, T] bf16
    # chunked layout [tci, p, ko, t']: per-partition contiguous DMA runs
    hidc = np.ascontiguousarray(
        hidT_b.reshape(KO, 128, T // TCH, TCH).transpose(2, 1, 0, 3)
    )
    hid8c = (hidc.astype(np.float32) * QK8_SCALE).astype(F8)
    ones3 = np.ones((3, 128), dtype=BF16)
    amq = am[0]
    # transposed diagonal blocks for the sT[k, q] score layout
    maskd = np.zeros((QT, 128, 128), np.float32)
    for qi in range(QT):
        blk = amq[qi * 128 : (qi + 1) * 128, qi * 128 : (qi + 1) * 128]
        maskd[qi] = np.where(blk, MASKVAL, 0.0).T

    in_maps = []
    for c in range(NCORES):
        heads = [HPC * c + i for i in range(HPC)]
        qk_cols = []
        bqk_c = np.empty((128, CT_QK), np.float32)
        for i, h in enumerate(heads):
            o = h * 3 * HD
            qk_cols.append(wqkv[:, o : o + HD] * (INV * QK8_SCALE))
            qk_cols.append(wqkv[:, o + HD : o + 2 * HD] * QK8_SCALE)
            bqk_c[:, 2 * i] = bqkv[o : o + HD] * INV
            bqk_c[:, 2 * i + 1] = bqkv[o + HD : o + 2 * HD]
        wqk_c = np.concatenate(qk_cols, axis=1).astype(F8)
        wqk_c = np.ascontiguousarray(
            wqk_c.reshape(KO, 128, CT_QK // 2, 256).transpose(2, 1, 0, 3)
        )
        wv_c = np.concatenate(
            [wqkv[:, h * 3 * HD + 2 * HD : (h + 1) * 3 * HD] for h in heads], axis=1
        ).astype(BF16)
        wv_c = np.ascontiguousarray(wv_c.reshape(KO, 128, HPC * 128).transpose(1, 0, 2))
        bv = np.concatenate(
            [bqkv[h * 3 * HD + 2 * HD : (h + 1) * 3 * HD] for h in heads]
        ).astype(np.float32)
        bv_hi = bv.astype(BF16)
        bv_lo = (bv - bv_hi.astype(np.float32)).astype(BF16)
        bv2_c = np.stack([bv_hi, bv_lo])
        alibi_c = np.empty((ITEMS, S), np.float32)
        for it in range(ITEMS):
            b, hl = divmod(it, HPC)
            alibi_c[it] = al[b * NH + heads[hl], 0, :]
        negc_c = -(np.maximum.accumulate(alibi_c, axis=1) + 1.0).astype(np.float32)
        wd_c = wdn[c * HPC * HD : (c + 1) * HPC * HD].astype(BF16)
        wd_c = np.ascontiguousarray(
            wd_c.reshape(HPC, 128, H // 256, 256).transpose(2, 1, 0, 3)
        )

        in_maps.append(
            dict(
                hidc=hidc,
                hid8c=hid8c,
                wqk8c=wqk_c,
                wvc=wv_c,
                wdc=wd_c,
                bqk=bqk_c,
                bv2=bv2_c,
                ones3=ones3,
                alibik=alibi_c,
                negcr=negc_c,
                maskd=maskd,
            )
        )
    return in_maps


def finish(partials, residual, b_dense):
    """Sum per-core partial outputs and add bias + residual."""
    res = np.asarray(residual, np.float32)
    bdn = np.asarray(b_dense, np.float32)
    acc = np.zeros((H, T), np.float32)
    for p in partials:
        acc += np.asarray(p, np.float32)
    out = acc.T.reshape(B, S, H) + bdn[None, None, :] + res
    return out.astype(np.float32)


def kernel(
    hidden_states,
    residual,
    alibi,
    attention_mask,
    W_qkv,
    b_qkv,
    W_dense,
    b_dense,
    num_heads=NH,
):
    from concourse.bass_utils import run_bass_kernel_spmd

    assert int(num_heads) == NH
    in_maps = make_in_maps(
        hidden_states, alibi, attention_mask, W_qkv, b_qkv, W_dense
    )
    nc = _get_nc()
    results = run_bass_kernel_spmd(
        nc, in_maps, core_ids=list(range(NCORES))
    ).results
    return finish([r["outT"] for r in results], residual, b_dense)



# revision 7
# speedup vs baseline: 1.6074x; 1.6074x over previous
"""BLOOM attention block (fused QKV proj + causal alibi attention + dense
projection) on 8 Trainium2 NeuronCores.

Sharding: tensor-parallel over heads. Each core owns 4 of the 32 heads:
it computes those heads' Q/K/V projections (column-sharded W_qkv),
attention, and a partial dense output (row-sharded W_dense over the same
head channels). The host sums the 8 partial outputs and adds
b_dense + residual.

Device-side design notes:
  - The Q/K *and* V projections run in fp8 with perf_mode=DoubleRow: two
    128-row k-subtiles are packed per matmul, so the PE contracts 256
    rows/instruction at 2 MACs/cell/cycle (~1.8x the bf16 rate). fp8
    error on q.k only shifts softmax logits by ~1e-3 against an alibi
    scale of ~1e2; fp8 error on v (~1%) is within the 2e-2 gate.
  - Activations are kept transposed ([feature, token]) so every matmul
    contracts over the partition dim with no on-chip transposes. Scores
    are computed directly transposed (sT = kT.T @ qT) so exp() writes
    probs^T straight into SBUF for the PV matmul. 16-bit tensors are
    fp16 (not bf16): same engine rates, 8x finer mantissa.
  - Softmax needs no reduce_max: the exp shift is the host-precomputed
    -(running_max(alibi)+1) (softmax is shift invariant; |q.k/sqrt(hd)|<<1).
    alibi[k] is a per-partition scalar in the transposed layout; both are
    applied in exact fp32 by one DVE scalar_tensor_tensor per score chunk.
  - Row sums: each item pair accumulates into one [2,512] PSUM tile via
    2-column ones stationaries, so a single DVE reciprocal (8 cyc/elem,
    free-dim-bound) serves two items; 1/sum is partition-broadcast
    (GpSimd) and fused into the small ctx copy (DVE), exact fp32.
  - The whole program is emitted as one fine-grained interleave:
    attention for batch 0 overlaps projection chunks 2-3, attention for
    batch 1 overlaps the batch-0 dense matmuls, so the attention phase's
    DVE/ACT chains hide behind PE-dense stretches instead of stalling
    the PE (the baseline lost ~90us to this).
  - The causal mask is additive -30000 on the 128x128 diagonal blocks only;
    blocks strictly below the transposed diagonal are never computed.
  - Host-side DRAM layouts are pre-tiled so every big DMA reads 8-16 KiB
    per-partition-contiguous runs; DMA issue streams are split across the
    SP/ACT/GpSimd sequencers so slot-gated waits never block prefetches.
"""

import math

import numpy as np
import ml_dtypes

B, S, H, NH = 2, 1024, 4096, 32
HD = H // NH  # 128
T = B * S  # 2048 tokens
NCORES = 8
HPC = NH // NCORES  # 4 heads per core
INV = 1.0 / math.sqrt(HD)
F16 = np.float16
F8 = ml_dtypes.float8_e4m3
Q8_SCALE = 64.0  # fp8 range lift for hidden/W; descaled after the matmul
Q8_DESCALE = 1.0 / (Q8_SCALE * Q8_SCALE)
MASKVAL = -30000.0

KO = H // 128  # 32 contraction subtiles over the hidden dim
KO2 = KO // 2  # 16 DoubleRow pair-steps
TCH = 512  # token chunk in the projection phase
NCH = T // TCH  # 4 chunks
CT_QK = 2 * HPC  # 8 q/k channel tiles per core (q_h0,k_h0,q_h1,k_h1,...)
ITEMS = B * HPC  # 8 (batch, head) attention items per core
QT = S // 128  # 8 query tiles per item

# eT blocks (k_tile, q_tile) that the PV matmul reads but no transpose
# writes (strictly-above-diagonal inside each 512-wide q chunk).
ZERO_BLOCKS = [
    (kt, qi)
    for qc in range(2)
    for kt in range(4 * qc, 4 * qc + 4)
    for qi in range(4 * qc, 4 * qc + 4)
    if kt > qi
]

_cache: dict = {}


def _build_nc():
    """Build the (SPMD, per-core) Bass/Tile program. Same program runs on
    all 8 cores; only the input data differs per core."""
    import concourse.bass as bass
    import concourse.mybir as mybir
    import concourse.tile as tile
    from concourse import bacc

    dt = mybir.dt
    f32, f16, f8 = dt.float32, dt.float16, dt.float8e4
    AF = mybir.ActivationFunctionType
    DR = mybir.MatmulPerfMode.DoubleRow

    nc = bacc.Bacc("TRN2", debug=False, num_devices=NCORES)

    # pre-tiled (host-side) layouts: every DMA reads per-partition-contiguous
    # runs, which maximizes per-queue DMA throughput
    hid8c = nc.dram_tensor(
        "hid8c", [NCH, 128, KO, TCH], f8, kind="ExternalInput"
    ).ap()
    wqk8c = nc.dram_tensor(
        "wqk8c", [CT_QK // 2, 128, KO, 256], f8, kind="ExternalInput"
    ).ap()
    wv8c = nc.dram_tensor("wv8c", [128, KO, HPC * 128], f8, kind="ExternalInput").ap()
    wdc = nc.dram_tensor(
        "wdc", [H // 256, 128, HPC, 256], f16, kind="ExternalInput"
    ).ap()
    bqk = nc.dram_tensor("bqk", [128, CT_QK], f32, kind="ExternalInput").ap()
    bvr = nc.dram_tensor("bvr", [1, HPC * 128], f32, kind="ExternalInput").ap()
    ones4 = nc.dram_tensor("ones4", [128, 4], f16, kind="ExternalInput").ap()
    # additive score terms, exact fp32: alibi[k] is a per-partition scalar
    # in the transposed score layout; -(running_max(alibi[:q]) + 1) (the
    # static exp shift replacing a reduce_max) is partition-broadcast.
    alibik = nc.dram_tensor("alibik", [ITEMS, S], f32, kind="ExternalInput").ap()
    negcr = nc.dram_tensor("negcr", [ITEMS, S], f32, kind="ExternalInput").ap()
    # transposed causal diagonal blocks (additive MASKVAL)
    maskd = nc.dram_tensor("maskd", [QT, 128, 128], f16, kind="ExternalInput").ap()
    outT = nc.dram_tensor("outT", [H, T], f16, kind="ExternalOutput").ap()

    maskd3 = maskd.rearrange("q p k -> p q k")

    with tile.TileContext(nc) as tc:
        with (
            tc.tile_pool(name="consts", bufs=1) as consts,
            tc.tile_pool(name="persist", bufs=1) as persist,
            tc.tile_pool(name="wvp", bufs=1) as wvp,
            tc.tile_pool(name="alp", bufs=3) as alp,
            tc.tile_pool(name="ncp", bufs=2) as ncp,
            tc.tile_pool(name="etp", bufs=2) as etp,
            tc.tile_pool(name="rsp", bufs=2) as rsp,
            tc.tile_pool(name="rcp", bufs=2) as rcp,
            tc.tile_pool(name="psS", bufs=2, space="PSUM") as psS,
            tc.tile_pool(name="psE", bufs=1, space="PSUM") as psE,
            tc.tile_pool(name="psPV", bufs=2, space="PSUM") as psPV,
        ):
            bqk_sb = consts.tile([128, CT_QK], f32, tag="bqk")
            nc.gpsimd.dma_start(bqk_sb, bqk)
            bvr_sb = consts.tile([1, HPC * 128], f32, tag="bvr")
            nc.gpsimd.dma_start(bvr_sb, bvr)
            bvb_sb = consts.tile([128, HPC * 128], f32, tag="bvb")
            nc.gpsimd.partition_broadcast(bvb_sb, bvr_sb)
            ones4_sb = consts.tile([128, 4], f16, tag="ones4")
            nc.gpsimd.dma_start(ones4_sb, ones4)
            maskd_sb = consts.tile([128, QT, 128], f16, tag="maskd")
            nc.gpsimd.dma_start(maskd_sb, maskd3)

            # Long-lived per-core activations.
            qkT_t = persist.tile([128, CT_QK, T], f16, tag="qkT")
            v_t = persist.tile([128, T // 128, HPC * 128], f16, tag="v")
            ctxT_t = persist.tile([128, HPC, T], f16, tag="ctxT")
            wv8_sb = wvp.tile([128, KO, HPC * 128], f8, tag="wv8")

            state: dict = {}

            # ---- projection phase generator: per chunk, the Q/K projection
            # ([ch, token] layout, weights stationary) and the V projection
            # ([token, ch] layout, hidden stationary), both fp8 DoubleRow.
            def proj_gen(hidp, wqkp, psA):
                for tci in range(NCH):
                    hid8 = hidp.tile([128, KO, TCH], f8, tag="hid8")
                    nc.sync.dma_start(hid8, hid8c[tci])
                    for cp in range(CT_QK // 2):
                        wqk8 = wqkp.tile([128, KO, 256], f8, tag="wqk8")
                        nc.scalar.dma_start(wqk8, wqk8c[cp])
                        for half in range(2):
                            ct = 2 * cp + half
                            ps = psA.tile([128, TCH], f32, tag="mm")
                            for k2 in range(KO2):
                                nc.tensor.matmul(
                                    ps,
                                    wqk8[:, 2 * k2 : 2 * k2 + 2, half * 128 : (half + 1) * 128],
                                    hid8[:, 2 * k2 : 2 * k2 + 2, :],
                                    start=(k2 == 0),
                                    stop=(k2 == KO2 - 1),
                                    perf_mode=DR,
                                )
                            # descale fp8 product + bias-add + f16 cast (DVE)
                            nc.vector.tensor_scalar(
                                out=qkT_t[:, ct, tci * TCH : (tci + 1) * TCH],
                                in0=ps,
                                scalar1=Q8_DESCALE,
                                scalar2=bqk_sb[:, ct : ct + 1],
                                op0=mybir.AluOpType.mult,
                                op1=mybir.AluOpType.add,
                            )
                            yield
                    if tci == 0:
                        # deferred: needed only by the V matmuls below
                        nc.scalar.dma_start(wv8_sb, wv8c)
                    for tt in range(TCH // 128):
                        ps = psA.tile([128, HPC * 128], f32, tag="mm")
                        for k2 in range(KO2):
                            nc.tensor.matmul(
                                ps,
                                hid8[:, 2 * k2 : 2 * k2 + 2, tt * 128 : (tt + 1) * 128],
                                wv8_sb[:, 2 * k2 : 2 * k2 + 2, :],
                                start=(k2 == 0),
                                stop=(k2 == KO2 - 1),
                                perf_mode=DR,
                            )
                        # descale + bias (broadcast along ch) + f16 cast
                        nc.vector.scalar_tensor_tensor(
                            out=v_t[:, tci * (TCH // 128) + tt, :],
                            in0=ps,
                            scalar=Q8_DESCALE,
                            in1=bvb_sb,
                            op0=mybir.AluOpType.mult,
                            op1=mybir.AluOpType.add,
                        )
                        yield

            # ---- attention: per (batch, head) item; items processed in
            # pairs so row-sum reciprocals batch 2 partitions per DVE op.
            def chunks_of(kt):
                q0 = kt * 128
                if q0 < 512:
                    return [(q0, 512), (512, S)]
                return [(q0, S)]

            def item_setup(it):
                b, hl = divmod(it, HPC)
                alik = alp.tile([128, QT], f32, tag="alik")
                nc.scalar.dma_start(
                    alik, alibik[it].rearrange("(kt p) -> p kt", p=128)
                )
                ncrow = alp.tile([1, S], f32, tag="ncrow")
                nc.scalar.dma_start(ncrow, negcr[it][None, :])
                ncb = ncp.tile([128, S], f32, tag="ncb")
                nc.gpsimd.partition_broadcast(ncb, ncrow)
                eT = etp.tile([128, QT, S], f16, tag="eT")
                for kt, qi in ZERO_BLOCKS:
                    nc.gpsimd.memset(eT[:, kt, qi * 128 : (qi + 1) * 128], 0.0)
                state[it] = dict(b=b, hl=hl, alik=alik, ncb=ncb, eT=eT)

            def score_chunk(it, kt):
                st = state[it]
                b, hl = st["b"], st["hl"]
                qTh = qkT_t[:, 2 * hl, b * S : (b + 1) * S]
                kTh = qkT_t[:, 2 * hl + 1, b * S : (b + 1) * S]
                eT = st["eT"]
                for ci, (q0, q1) in enumerate(chunks_of(kt)):
                    ps = psS.tile([128, 512], f32, tag="s")
                    nc.tensor.matmul(
                        ps[:, : q1 - q0],
                        kTh[:, kt * 128 : (kt + 1) * 128],
                        qTh[:, q0:q1],
                        start=True,
                        stop=True,
                    )
                    # score += alibi[k] (per-partition) + negc[q] (bcast)
                    nc.vector.scalar_tensor_tensor(
                        out=ps[:, : q1 - q0],
                        in0=ps[:, : q1 - q0],
                        scalar=st["alik"][:, kt : kt + 1],
                        in1=st["ncb"][:, q0:q1],
                        op0=mybir.AluOpType.add,
                        op1=mybir.AluOpType.add,
                    )
                    if ci == 0:  # causal diagonal block: first 128 cols
                        nc.vector.tensor_add(
                            ps[:, :128], ps[:, :128], maskd_sb[:, kt, :]
                        )
                    nc.scalar.activation(
                        eT[:, kt, q0:q1],
                        ps[:, : q1 - q0],
                        AF.Exp,
                        bias=0.0,
                        scale=1.0,
                    )

            def rowsum_qc(pair, qc):
                # both items of the pair accumulate into one [2, 512] bank;
                # partition j holds item j's row sums.
                ktn = 4 * (qc + 1)
                ps = psE.tile([2, 512], f32, tag="rs")
                for j, it in enumerate(pair):
                    eT = state[it]["eT"]
                    for kt in range(ktn):
                        nc.tensor.matmul(
                            ps,
                            ones4_sb[:, 2 * j : 2 * j + 2],
                            eT[:, kt, qc * 512 : (qc + 1) * 512],
                            start=(j == 0 and kt == 0),
                            stop=(j == 1 and kt == ktn - 1),
                        )
                rs = rsp.tile([2, 512], f32, tag="rs")
                nc.vector.reciprocal(rs, ps)
                # partition_broadcast sources must start at partition 0:
                # move item B's row down via a tiny SBUF->SBUF DMA.
                rs1 = rsp.tile([1, 512], f32, tag="rs1")
                nc.gpsimd.dma_start(rs1, rs[1:2, :])
                for j, it in enumerate(pair):
                    st = state[it]
                    if "rcb" not in st:
                        rcb = rcp.tile([128, S], f32, tag="rcb")
                        st["rcb"] = rcb
                    nc.gpsimd.partition_broadcast(
                        st["rcb"][:, qc * 512 : (qc + 1) * 512],
                        rs[0:1, :] if j == 0 else rs1,
                    )

            def pv_qc(it, qc):
                st = state[it]
                b, hl, eT = st["b"], st["hl"], st["eT"]
                ktn = 4 * (qc + 1)
                ps = psPV.tile([128, 512], f32, tag="mm")
                for kt in range(ktn):
                    nc.tensor.matmul(
                        ps,
                        v_t[:, b * 8 + kt, hl * 128 : (hl + 1) * 128],
                        eT[:, kt, qc * 512 : (qc + 1) * 512],
                        start=(kt == 0),
                        stop=(kt == ktn - 1),
                    )
                # fused 1/rowsum normalization + f16 cast
                nc.vector.tensor_tensor(
                    out=ctxT_t[:, hl, b * S + qc * 512 : b * S + (qc + 1) * 512],
                    in0=ps,
                    in1=st["rcb"][:, qc * 512 : (qc + 1) * 512],
                    op=mybir.AluOpType.mult,
                )

            def pair_gen(g):
                pair = (2 * g, 2 * g + 1)
                item_setup(pair[0])
                yield
                for kt in range(0, QT, 2):
                    score_chunk(pair[0], kt)
                    score_chunk(pair[0], kt + 1)
                    yield
                item_setup(pair[1])
                yield
                for kt in range(0, QT, 2):
                    score_chunk(pair[1], kt)
                    score_chunk(pair[1], kt + 1)
                    yield
                rowsum_qc(pair, 0)
                yield
                rowsum_qc(pair, 1)
                yield
                pv_qc(pair[0], 0)
                pv_qc(pair[0], 1)
                state.pop(pair[0])
                yield
                pv_qc(pair[1], 0)
                pv_qc(pair[1], 1)
                state.pop(pair[1])
                yield

            # ---- dense partial: outT[o, t] = sum_c Wd[c, o] ctx[t, c],
            # one generator per batch half so it can interleave with the
            # other batch's attention.
            def dense_gen(b, wdp, outp, psD):
                for op_ in range(H // 256):
                    wdt = wdp.tile([128, HPC, 256], f16, tag=f"wd{b}")
                    nc.scalar.dma_start(wdt, wdc[op_])
                    for half in range(2):
                        ot = 2 * op_ + half
                        ob = outp.tile([128, S], f16, tag=f"ob{b}")
                        for tcd in range(2):
                            ps = psD.tile([128, 512], f32, tag="mm")
                            for ko in range(HPC):
                                nc.tensor.matmul(
                                    ps,
                                    wdt[:, ko, half * 128 : (half + 1) * 128],
                                    ctxT_t[:, ko, b * S + tcd * 512 : b * S + (tcd + 1) * 512],
                                    start=(ko == 0),
                                    stop=(ko == HPC - 1),
                                )
                            # alternate psum-evacuation between DVE and ACT
                            dst = ob[:, tcd * 512 : (tcd + 1) * 512]
                            if (op_ + half + tcd) % 2 == 0:
                                nc.vector.tensor_copy(out=dst, in_=ps)
                            else:
                                nc.scalar.activation(dst, ps, AF.Copy)
                        for hh in range(2):
                            nc.sync.dma_start(
                                outT[
                                    ot * 128 : (ot + 1) * 128,
                                    b * S + hh * 512 : b * S + (hh + 1) * 512,
                                ],
                                ob[:, hh * 512 : (hh + 1) * 512],
                            )
                        yield

            def drain(gen, n=None):
                if n is None:
                    for _ in gen:
                        pass
                else:
                    for _ in range(n):
                        next(gen, None)

            pairs = [pair_gen(g) for g in range(ITEMS // 2)]

            def next_pair(cands):
                # advance the first non-exhausted pair generator; pairs run
                # strictly sequentially (a 2-deep eT/ncb/rcb ring means a
                # later pair's setup waits on an earlier pair's last reads —
                # overlapping pairs would deadlock the PE FIFO).
                for p in cands:
                    if next(p, StopIteration) is not StopIteration:
                        return

            with (
                tc.tile_pool(name="hidp", bufs=2) as hidp,
                tc.tile_pool(name="wqkp", bufs=2) as wqkp,
                tc.tile_pool(name="psA", bufs=2, space="PSUM") as psA,
            ):
                pj = proj_gen(hidp, wqkp, psA)
                drain(pj, 24)  # chunks 0, 1 (batch 0)
                # chunks 2, 3 interleaved with batch-0 attention (pairs 0, 1)
                for i in range(24):
                    next(pj, None)
                    next_pair(pairs[:2])
                drain(pj)
            drain(pairs[0])
            drain(pairs[1])

            with (
                tc.tile_pool(name="wdp", bufs=3) as wdp,
                tc.tile_pool(name="outp", bufs=4) as outp,
                tc.tile_pool(name="psD", bufs=3, space="PSUM") as psD,
            ):
                d0 = dense_gen(0, wdp, outp, psD)
                d1 = dense_gen(1, wdp, outp, psD)
                # batch-1 attention (pairs 2, 3) interleaved with batch-0
                # dense.
                for i in range(32):
                    next(d0, None)
                    next_pair(pairs[2:])
                drain(pairs[2])
                drain(pairs[3])
                drain(d1)
    nc.compile()
    return nc


def _get_nc():
    if "nc" not in _cache:
        _cache["nc"] = _build_nc()
    return _cache["nc"]


def make_in_maps(
    hidden_states, alibi, attention_mask, W_qkv, b_qkv, W_dense
) -> list[dict]:
    """Host-side sharding/preprocessing: per-core input dicts."""
    hs = np.asarray(hidden_states, np.float32)
    al = np.asarray(alibi, np.float32)
    am = np.asarray(attention_mask).astype(bool)
    wqkv = np.asarray(W_qkv, np.float32)
    bqkv = np.asarray(b_qkv, np.float32)
    wdn = np.asarray(W_dense, np.float32)

    def to_f8(x):
        return np.clip(x * Q8_SCALE, -240.0, 240.0).astype(F8)

    hidT = hs.reshape(T, H).T  # [H, T] fp32
    # chunked layout [tci, p, ko, t']: per-partition contiguous DMA runs
    hid8c = np.ascontiguousarray(
        to_f8(hidT).reshape(KO, 128, NCH, TCH).transpose(2, 1, 0, 3)
    )
    ones4 = np.zeros((128, 4), dtype=F16)
    ones4[:, 0] = 1.0
    ones4[:, 3] = 1.0
    amq = am[0]
    # transposed diagonal blocks for the sT[k, q] score layout
    maskd = np.zeros((QT, 128, 128), F16)
    for qi in range(QT):
        blk = amq[qi * 128 : (qi + 1) * 128, qi * 128 : (qi + 1) * 128]
        maskd[qi] = np.where(blk, MASKVAL, 0.0).T

    in_maps = []
    for c in range(NCORES):
        heads = [HPC * c + i for i in range(HPC)]
        qk_cols = []
        bqk_c = np.empty((128, CT_QK), np.float32)
        for i, h in enumerate(heads):
            o = h * 3 * HD
            qk_cols.append(wqkv[:, o : o + HD] * INV)
            qk_cols.append(wqkv[:, o + HD : o + 2 * HD])
            bqk_c[:, 2 * i] = bqkv[o : o + HD] * INV
            bqk_c[:, 2 * i + 1] = bqkv[o + HD : o + 2 * HD]
        wqk_c = to_f8(np.concatenate(qk_cols, axis=1))
        wqk_c = np.ascontiguousarray(
            wqk_c.reshape(KO, 128, CT_QK // 2, 256).transpose(2, 1, 0, 3)
        )
        wv_c = to_f8(
            np.concatenate(
                [wqkv[:, h * 3 * HD + 2 * HD : (h + 1) * 3 * HD] for h in heads],
                axis=1,
            )
        )
        wv_c = np.ascontiguousarray(wv_c.reshape(KO, 128, HPC * 128).transpose(1, 0, 2))
        bv_c = np.concatenate(
            [bqkv[h * 3 * HD + 2 * HD : (h + 1) * 3 * HD] for h in heads]
        ).astype(np.float32)[None, :]
        alibi_c = np.empty((ITEMS, S), np.float32)
        for it in range(ITEMS):
            b, hl = divmod(it, HPC)
            alibi_c[it] = al[b * NH + heads[hl], 0, :]
        negc_c = -(np.maximum.accumulate(alibi_c, axis=1) + 1.0).astype(np.float32)
        wd_c = wdn[c * HPC * HD : (c + 1) * HPC * HD].astype(F16)
        wd_c = np.ascontiguousarray(
            wd_c.reshape(HPC, 128, H // 256, 256).transpose(2, 1, 0, 3)
        )

        in_maps.append(
            dict(
                hid8c=hid8c,
                wqk8c=wqk_c,
                wv8c=wv_c,
                wdc=wd_c,
                bqk=bqk_c,
                bvr=bv_c,
                ones4=ones4,
                alibik=alibi_c,
                negcr=negc_c,
                maskd=maskd,
            )
        )
    return in_maps


def finish(partials, residual, b_dense):
    """Sum per-core partial outputs and add bias + residual."""
    res = np.asarray(residual, np.float32)
    bdn = np.asarray(b_dense, np.float32)
    acc = np.zeros((H, T), np.float32)
    for p in partials:
        acc += np.asarray(p, np.float32)
    out = acc.T.reshape(B, S, H) + bdn[None, None, :] + res
    return out.astype(np.float32)


def kernel(
    hidden_states,
    residual,
    alibi,
    attention_mask,
    W_qkv,
    b_qkv,
    W_dense,
    b_dense,
    num_heads=NH,
):
    from concourse.bass_utils import run_bass_kernel_spmd

    assert int(num_heads) == NH
    in_maps = make_in_maps(
        hidden_states, alibi, attention_mask, W_qkv, b_qkv, W_dense
    )
    nc = _get_nc()
    results = run_bass_kernel_spmd(
        nc, in_maps, core_ids=list(range(NCORES))
    ).results
    return finish([r["outT"] for r in results], residual, b_dense)
